# revision 24
# baseline (speedup 1.0000x reference)
"""Trainium2 Bass kernel for nn_LonelyDecoder (dense transformer, 8-core TP).

Key observations baked in:
 - In the reference, every layer recomputes from the embedding output `h`
   and only the LAST layer's `out` feeds the logits -> layers 0..L-2 are
   dead code. We compute: embedding GEMM, layer L-1, output GEMM+softmax.
 - Scores are tiny ((q.k)/1024, |s| < ~1), so softmax needs no max pass.
 - All activations are kept TRANSPOSED ([feature, seq]) so that:
     * matmul contraction dims land on partitions for both operands,
     * per-feature biases are per-partition (fused into ACT evictions),
     * head-concat == AllGather along the partition axis.

Sharding (8 cores):
 - vocab dim of x/emb_W/out_W (4000/core, padded to 4096)
 - heads of attention (2 heads/core), DFF of the FFN (512/core)

v2 restructure (vs v1): collectives are bf16, chunked along S, and
issued per-chunk so they overlap the producing GEMMs and downstream
compute; the tail (FFN -> yAR -> LN -> out GEMM -> softmax) is a per-
s-chunk pipeline with the exp values round-tripped through DRAM but
overlapped; LN is fused (residual+stats+normalize, bf16 vector ops,
scalar-engine per-partition affine).
"""

import numpy as np
import ml_dtypes

import concourse.bacc as bacc
import concourse.bass as bass
import concourse.mybir as mybir
import concourse.tile as tile
from concourse.bass_utils import run_bass_kernel_spmd

F32 = mybir.dt.float32
BF16 = mybir.dt.bfloat16
AF = mybir.ActivationFunctionType
ALU = mybir.AluOpType

S, V, D, H, DK, DFF, L = 2048, 32000, 1024, 16, 64, 4096, 4
NCORES = 8
VSR = V // NCORES          # 4000 real vocab shard
VSP = 4096                 # padded vocab shard (32 x 128)
NVC = VSP // 128           # 32 v-chunks
NDC = D // 128             # 8 d-chunks
NSC = 4                    # s-chunks of 512
SC = 512
NTT = S // 128             # 16 t-tiles
FS = DFF // NCORES         # 512 ff shard
NFC = FS // 128            # 4 ff chunks
RG = [list(range(NCORES))]

# collective chunking: merge factor in s-chunks (1, 2, or 4)
HMERGE = 2                 # embedding-output AllReduce
AMERGE = 2                 # attention-output AllGathers
YMERGE = 2                 # FFN-output AllReduce
SSMERGE = 2                # softmax-sum AllReduce

LAST_RESULTS = {}          # stash for test harness (exec time etc.)


def ts(i, n):
    return slice(i * n, (i + 1) * n)


def build_bass():
    nc = bacc.Bacc(None, target_bir_lowering=False)

    # ---- I/O ----
    xT = nc.dram_tensor("xT", [VSP, S], BF16, kind="ExternalInput")
    embW = nc.dram_tensor("embW", [VSP, D], BF16, kind="ExternalInput")
    pebT = nc.dram_tensor("pebT", [D, S], BF16, kind="ExternalInput")
    qkw = [nc.dram_tensor(f"qkw{m}", [128, NDC, 256], BF16, kind="ExternalInput") for m in (1, 2)]
    bqk = [nc.dram_tensor(f"bqk{m}", [128, 2], F32, kind="ExternalInput") for m in (1, 2)]
    vw = [nc.dram_tensor(f"vw{m}", [128, NDC, 130], BF16, kind="ExternalInput") for m in (1, 2)]
    bv = [nc.dram_tensor(f"bv{m}", [128, 1], F32, kind="ExternalInput") for m in (1, 2)]
    maskT = nc.dram_tensor("maskT", [128, 4 * SC], BF16, kind="ExternalInput")
    f1w = nc.dram_tensor("f1w", [128, NDC, FS], BF16, kind="ExternalInput")
    f1b = nc.dram_tensor("f1b", [128, NFC], F32, kind="ExternalInput")
    f2w = nc.dram_tensor("f2w", [128, NFC, D], BF16, kind="ExternalInput")
    f2bT = nc.dram_tensor("f2bT", [128, NDC], F32, kind="ExternalInput")
    lngT = nc.dram_tensor("lngT", [128, NDC], F32, kind="ExternalInput")
    lnbT = nc.dram_tensor("lnbT", [128, NDC], F32, kind="ExternalInput")
    outw = nc.dram_tensor("outw", [128, NDC, VSP], BF16, kind="ExternalInput")
    outb = nc.dram_tensor("outb", [128, NVC], F32, kind="ExternalInput")
    probsT = nc.dram_tensor("probsT", [VSP, S], F32, kind="ExternalOutput")

    with tile.TileContext(nc) as tc:
        with tc.tile_pool(name="dram", bufs=1, space="DRAM") as dram, \
             tc.tile_pool(name="ps", bufs=8, space="PSUM") as ps, \
             tc.tile_pool(name="const", bufs=1) as const, \
             tc.tile_pool(name="acts", bufs=2) as acts, \
             tc.tile_pool(name="ev", bufs=3) as evp, \
             tc.tile_pool(name="small", bufs=2) as smp:

            # internal DRAM; chunked layouts: s-chunk sc lives at rows
            # [sc*D:(sc+1)*D] (or sc*128 for per-core attn out) so each
            # collective chunk is a contiguous row block.
            # Shared (collective-output) tensors must each have a single
            # writing instruction -> one tensor per collective chunk group.
            h_par = dram.tile([NSC * D, SC], BF16, tag="h_par")
            h_red = [dram.tile([HMERGE * D, SC], BF16, tag=f"h_red{g}",
                               addr_space="Shared", name=f"h_red{g}")
                     for g in range(NSC // HMERGE)]
            a_in = [dram.tile([NSC * 128, SC], BF16, tag=f"a{m}_in", name=f"a{m}_in") for m in (1, 2)]
            a_out = [[dram.tile([AMERGE * D, SC], BF16, tag=f"a{m}_out{g}",
                                addr_space="Shared", name=f"a{m}_out{g}")
                      for g in range(NSC // AMERGE)] for m in (1, 2)]
            y_par = dram.tile([NSC * D, SC], BF16, tag="y_par")
            y_red = [dram.tile([YMERGE * D, SC], BF16, tag=f"y_red{g}",
                               addr_space="Shared", name=f"y_red{g}")
                     for g in range(NSC // YMERGE)]
            e_dram = dram.tile([VSP, S], BF16, tag="e_dram")
            ss_in = dram.tile([1, S], F32, tag="ss_in")
            ss_out = [dram.tile([1, SSMERGE * SC], F32, tag=f"ss_out{g}",
                                addr_space="Shared", name=f"ss_out{g}")
                      for g in range(NSC // SSMERGE)]

            # row of feature-block dc for s-chunk sc inside its AllGather
            # group's output (out rows = core-major, chunk-minor)
            def ag_row(sc, dc):
                return dc * AMERGE * 128 + (sc % AMERGE) * 128

            # constants
            ones_bf_col = const.tile([128, 1], BF16, tag="c1")
            nc.vector.memset(ones_bf_col[:, :], 1.0)
            ones_row = const.tile([1, 128], F32, tag="c3")
            nc.vector.memset(ones_row[:, :], 1.0)
            ones_row64 = const.tile([1, 64], F32, tag="c4")
            nc.vector.memset(ones_row64[:, :], 1.0)
            eps_tile = const.tile([1, 1], F32, tag="c5")
            nc.vector.memset(eps_tile[:, :], 1e-5)
            mask_sb = const.tile([128, 4 * SC], BF16, tag="mask")
            nc.sync.dma_start(mask_sb[:, :], maskT[:, :])
            bqk_sb = [const.tile([128, 2], F32, tag=f"bqk{m}", name=f"bqk_sb{m}") for m in range(2)]
            bv_sb = [const.tile([128, 1], F32, tag=f"bv{m}", name=f"bv_sb{m}") for m in range(2)]
            for m in range(2):
                nc.sync.dma_start(bqk_sb[m][:, :], bqk[m][:, :])
                nc.sync.dma_start(bv_sb[m][:, :], bv[m][:, :])
            f1b_sb = const.tile([128, NFC], F32, tag="f1b")
            nc.sync.dma_start(f1b_sb[:, :], f1b[:, :])
            f2bT_sb = const.tile([128, NDC], F32, tag="f2bT")
            nc.sync.dma_start(f2bT_sb[:, :], f2bT[:, :])
            lng_sb = const.tile([128, NDC], F32, tag="lng")
            nc.sync.dma_start(lng_sb[:, :], lngT[:, :])
            lnb_sb = const.tile([128, NDC], F32, tag="lnb")
            nc.sync.dma_start(lnb_sb[:, :], lnbT[:, :])
            outb_sb = const.tile([128, NVC], F32, tag="outb")
            nc.sync.dma_start(outb_sb[:, :], outb[:, :])

            # ---------- embedding GEMM:  hT_partial = embW^T @ xT ----------
            # per s-chunk; AllReduce (bf16) issued per HMERGE chunks so it
            # overlaps the next chunk's GEMM.
            with tc.tile_pool(name="embw", bufs=1) as embp, \
                 tc.tile_pool(name="xt", bufs=4) as xtp:
                embw_sb = embp.tile([128, NVC, D], BF16, tag="embw")
                for kc in range(NVC):
                    nc.sync.dma_start(embw_sb[:, kc, :], xT_rows(embW, kc))
                for sc in range(NSC):
                    pes = []
                    for dc in range(NDC):
                        t = ps.tile([128, SC], F32, tag="ps", name=f"pe_{sc}_{dc}")
                        pes.append(t)
                    for kc in range(NVC):
                        xt = xtp.tile([128, SC], BF16, tag="xt")
                        nc.sync.dma_start(xt[:, :], xT[ts(kc, 128), ts(sc, SC)])
                        for dc in range(NDC):
                            nc.tensor.matmul(
                                pes[dc][:, :],
                                embw_sb[:, kc, ts(dc, 128)],
                                xt[:, :],
                                start=(kc == 0),
                                stop=(kc == NVC - 1),
                            )
                    for dc in range(NDC):
                        hv = evp.tile([128, SC], BF16, tag="ev")
                        nc.scalar.activation(hv[:, :], pes[dc][:, :], AF.Copy)
                        nc.sync.dma_start(h_par[ts(sc * NDC + dc, 128), :], hv[:, :])
                    if (sc + 1) % HMERGE == 0:
                        g0 = sc + 1 - HMERGE
                        nc.gpsimd.collective_compute(
                            "AllReduce", ALU.add, replica_groups=RG,
                            ins=[h_par[g0 * D:(sc + 1) * D, :].opt()],
                            outs=[h_red[sc // HMERGE][:, :].opt()],
                        )

            # ======== phase A: attention x2, layernorms, FFN ========
            with tc.tile_pool(name="addin", bufs=4) as adp, \
                 tc.tile_pool(name="x2p", bufs=3) as x2p, \
                 tc.tile_pool(name="bcast", bufs=4) as bcp:

                # hT = h_red + (emb_b + PE)^T   -> bf16 resident
                hT = acts.tile([128, NDC, S], BF16, tag="act", name="hT")
                for sc in range(NSC):
                    for dc in range(NDC):
                        hr = adp.tile([128, SC], BF16, tag="addin", name=f"hr_{sc}_{dc}")
                        nc.sync.dma_start(
                            hr[:, :],
                            h_red[sc // HMERGE][ts((sc % HMERGE) * NDC + dc, 128), :])
                        pb = adp.tile([128, SC], BF16, tag="addin", name=f"pb_{sc}_{dc}")
                        nc.sync.dma_start(pb[:, :], pebT[ts(dc, 128), ts(sc, SC)])
                        nc.vector.tensor_add(hT[:, dc, ts(sc, SC)], hr[:, :], pb[:, :])

                # fused residual + layernorm for one s-chunk:
                #   r(sc) = prevT(sc) + addin(sc)   (in place into prevT)
                #   newT(sc) = (r - mu) * inv * g + b
                def ln_chunk(prevT, newT, sc, load_addin, name, extra_bias=None):
                    for dc in range(NDC):
                        ad = adp.tile([128, SC], BF16, tag="addin",
                                      name=f"ad_{name}_{sc}_{dc}")
                        nc.sync.dma_start(ad[:, :], load_addin(dc))
                        if extra_bias is not None:
                            ab = adp.tile([128, SC], BF16, tag="addin",
                                          name=f"ab_{name}_{sc}_{dc}")
                            nc.scalar.activation(ab[:, :], ad[:, :], AF.Identity,
                                                 bias=extra_bias[:, dc:dc + 1])
                            ad = ab
                        nc.vector.tensor_add(prevT[:, dc, ts(sc, SC)],
                                             prevT[:, dc, ts(sc, SC)], ad[:, :])
                    st = ps.tile([65, SC], F32, tag="ps", name=f"st_{name}_{sc}")
                    for dc in range(NDC):
                        x2 = x2p.tile([128, SC], BF16, tag="x2", name=f"x2_{name}_{sc}_{dc}")
                        nc.vector.tensor_mul(x2[:, :], prevT[:, dc, ts(sc, SC)],
                                             prevT[:, dc, ts(sc, SC)])
                        nc.tensor.matmul(st[0:1, :], ones_bf_col[:, :],
                                         prevT[:, dc, ts(sc, SC)],
                                         start=(dc == 0), stop=(dc == NDC - 1))
                        nc.tensor.matmul(st[64:65, :], ones_bf_col[:, :], x2[:, :],
                                         start=(dc == 0), stop=(dc == NDC - 1))
                    nm = smp.tile([1, SC], F32, tag="nm", name=f"nm_{name}_{sc}")
                    nc.vector.tensor_scalar_mul(nm[:, :], st[0:1, :], -1.0 / D)
                    e2 = smp.tile([1, SC], F32, tag="e2", name=f"e2_{name}_{sc}")
                    nc.vector.tensor_scalar_mul(e2[:, :], st[64:65, :], 1.0 / D)
                    musq = smp.tile([1, SC], F32, tag="scr2", name=f"mq_{name}_{sc}")
                    nc.vector.tensor_mul(musq[:, :], nm[:, :], nm[:, :])
                    nc.vector.tensor_sub(e2[:, :], e2[:, :], musq[:, :])
                    nc.scalar.activation(e2[:, :], e2[:, :], AF.Sqrt, bias=eps_tile[:, :])
                    inv = smp.tile([1, SC], F32, tag="inv1", name=f"inv_{name}_{sc}")
                    nc.vector.reciprocal(inv[:, :], e2[:, :])
                    nminv = smp.tile([1, SC], F32, tag="scr2", name=f"nmi_{name}_{sc}")
                    nc.vector.tensor_mul(nminv[:, :], nm[:, :], inv[:, :])
                    # broadcast inv and nm*inv over partitions via PE
                    pi = ps.tile([128, SC], F32, tag="ps", name=f"pi_{name}_{sc}")
                    nc.tensor.matmul(pi[:, :], ones_row[:, :], inv[:, :],
                                     start=True, stop=True)
                    inv_b = bcp.tile([128, SC], BF16, tag="bc", name=f"invb_{name}_{sc}")
                    nc.scalar.activation(inv_b[:, :], pi[:, :], AF.Copy)
                    pm = ps.tile([128, SC], F32, tag="ps", name=f"pm_{name}_{sc}")
                    nc.tensor.matmul(pm[:, :], ones_row[:, :], nminv[:, :],
                                     start=True, stop=True)
                    nmi_b = bcp.tile([128, SC], BF16, tag="bc", name=f"nmib_{name}_{sc}")
                    nc.scalar.activation(nmi_b[:, :], pm[:, :], AF.Copy)
                    for dc in range(NDC):
                        t1 = x2p.tile([128, SC], BF16, tag="x2", name=f"t1_{name}_{sc}_{dc}")
                        nc.vector.tensor_mul(t1[:, :], prevT[:, dc, ts(sc, SC)], inv_b[:, :])
                        nc.vector.tensor_add(t1[:, :], t1[:, :], nmi_b[:, :])
                        nc.scalar.activation(newT[:, dc, ts(sc, SC)], t1[:, :], AF.Identity,
                                             scale=lng_sb[:, dc:dc + 1],
                                             bias=lnb_sb[:, dc:dc + 1])

                with tc.tile_pool(name="attn", bufs=1) as attnp:
                    # --- mha1 (masked), pipelined per s-chunk ---
                    qkw_sb = attnp.tile([128, NDC, 256], BF16, tag="qkw", name="qkw_sb0")
                    nc.sync.dma_start(qkw_sb[:, :, :], qkw[0][:, :, :])
                    vw_sb = attnp.tile([128, NDC, 130], BF16, tag="vw", name="vw_sb0")
                    nc.sync.dma_start(vw_sb[:, :, :], vw[0][:, :, :])
                    V_sb = attnp.tile([128, NTT, 130], BF16, tag="V", name="V_sb0")
                    qT2 = attnp.tile([128, NSC, SC], BF16, tag="qT2", name="qT2_0")
                    kT2 = attnp.tile([128, NSC, SC], BF16, tag="kT2", name="kT2_0")
                    attnT = attnp.tile([128, NSC, SC], BF16, tag="attnT", name="attnT0")

                    def qkv_proj(mi, actT, sc, V_sb, qT2, kT2, qkw_sb, vw_sb):
                        for tt in range(4 * sc, 4 * sc + 4):
                            pv = ps.tile([128, 130], F32, tag="ps", name=f"pv{mi}_{tt}")
                            for dc in range(NDC):
                                nc.tensor.matmul(
                                    pv[:, :], actT[:, dc, ts(tt, 128)], vw_sb[:, dc, :],
                                    start=(dc == 0), stop=(dc == NDC - 1),
                                )
                            nc.scalar.activation(V_sb[:, tt, :], pv[:, :], AF.Copy)
                        nc.vector.memset(V_sb[:, ts(sc, 4), 64:65], 1.0)
                        nc.vector.memset(V_sb[:, ts(sc, 4), 129:130], 1.0)
                        for wi, dst in ((0, qT2), (1, kT2)):
                            pq = ps.tile([128, SC], F32, tag="ps", name=f"pq{mi}_{wi}_{sc}")
                            for dc in range(NDC):
                                nc.tensor.matmul(
                                    pq[:, :],
                                    qkw_sb[:, dc, ts(wi, 128)],
                                    actT[:, dc, ts(sc, SC)],
                                    start=(dc == 0), stop=(dc == NDC - 1),
                                )
                            nc.scalar.activation(
                                dst[:, sc, :], pq[:, :], AF.Identity,
                                bias=bqk_sb[mi][:, wi:wi + 1],
                            )

                    def attn_chunk(mi, sc, tts, V_sb, qT2, kT2, attnT):
                        """scores+AV+normalize for query chunk sc over key tiles
                        tts; writes attnT[:, sc, :] and a_in chunk, AGs on merge."""
                        masked = (mi == 0)
                        for h in range(2):
                            po = ps.tile([128, SC], F32, tag="ps", name=f"po{mi}_{h}_{sc}")
                            for i, tt in enumerate(tts):
                                pscr = ps.tile([128, SC], F32, tag="ps",
                                               name=f"pscr{mi}_{h}_{sc}_{tt}")
                                nc.tensor.matmul(
                                    pscr[:, :],
                                    kT2[ts(h, 64), tt // 4, ts(tt % 4, 128)],
                                    qT2[ts(h, 64), sc, :],
                                    start=True, stop=True,
                                )
                                et = evp.tile([128, SC], BF16, tag="exp")
                                nc.scalar.activation(et[:, :], pscr[:, :], AF.Exp,
                                                     scale=1.0 / D)
                                if masked and tt >= 4 * sc:
                                    nc.vector.tensor_mul(
                                        et[:, :], et[:, :],
                                        mask_sb[:, ts(tt - 4 * sc, SC)],
                                    )
                                nc.tensor.matmul(
                                    po[0:65, :],
                                    V_sb[:, tt, ts(h, 65)],
                                    et[:, :],
                                    start=(i == 0), stop=(i == len(tts) - 1),
                                )
                            rec = attnp.tile([1, SC], F32, tag="rec", bufs=2,
                                             name=f"rec{mi}_{h}_{sc}")
                            nc.vector.reciprocal(rec[:, :], po[64:65, :])
                            pbv = ps.tile([128, SC], F32, tag="ps", name=f"pb{mi}_{h}_{sc}")
                            nc.tensor.matmul(pbv[0:64, :], ones_row64[:, :], rec[:, :],
                                             start=True, stop=True)
                            oo = attnp.tile([64, SC], F32, tag="oo", bufs=2,
                                            name=f"oo{mi}_{h}_{sc}")
                            nc.scalar.activation(oo[:, :], po[0:64, :], AF.Copy)
                            tmp = attnp.tile([64, SC], F32, tag="avtmp", bufs=2,
                                             name=f"avtmp{mi}_{h}_{sc}")
                            nc.vector.tensor_mul(tmp[:, :], oo[:, :], pbv[0:64, :])
                            nc.scalar.activation(
                                attnT[ts(h, 64), sc, :], tmp[:, :], AF.Identity,
                                bias=bv_sb[mi][ts(h, 64), :],
                            )
                        nc.sync.dma_start(a_in[mi][ts(sc, 128), :], attnT[:, sc, :])
                        if (sc + 1) % AMERGE == 0:
                            g0 = sc + 1 - AMERGE
                            nc.gpsimd.collective_compute(
                                "AllGather", ALU.bypass, replica_groups=RG,
                                ins=[a_in[mi][g0 * 128:(sc + 1) * 128, :].opt()],
                                outs=[a_out[mi][sc // AMERGE][:, :].opt()],
                            )

                    h1T = acts.tile([128, NDC, S], BF16, tag="act", name="h1T")
                    for sc in range(NSC):
                        qkv_proj(0, hT, sc, V_sb, qT2, kT2, qkw_sb, vw_sb)
                        attn_chunk(0, sc, list(range(4 * (sc + 1))), V_sb, qT2, kT2, attnT)
                    # ln1 per chunk (waits on the AG covering that chunk)
                    qkw_sb2 = attnp.tile([128, NDC, 256], BF16, tag="qkw2", name="qkw_sb1")
                    nc.sync.dma_start(qkw_sb2[:, :, :], qkw[1][:, :, :])
                    vw_sb2 = attnp.tile([128, NDC, 130], BF16, tag="vw2", name="vw_sb1")
                    nc.sync.dma_start(vw_sb2[:, :, :], vw[1][:, :, :])
                    V_sb2 = attnp.tile([128, NTT, 130], BF16, tag="V2", name="V_sb1")
                    qT2b = attnp.tile([128, NSC, SC], BF16, tag="qT2b", name="qT2_1")
                    kT2b = attnp.tile([128, NSC, SC], BF16, tag="kT2b", name="kT2_1")
                    attnT2 = attnp.tile([128, NSC, SC], BF16, tag="attnT2", name="attnT1")
                    for sc in range(NSC):
                        ln_chunk(hT, h1T, sc,
                                 lambda dc, g=sc: a_out[0][g // AMERGE][
                                     ts(ag_row(g, dc) // 128, 128), :],
                                 "h1T")
                        # mha2 projections as soon as h1T chunk exists
                        qkv_proj(1, h1T, sc, V_sb2, qT2b, kT2b, qkw_sb2, vw_sb2)

                    # --- mha2 (unmasked): all K/V ready now ---
                    h2T = acts.tile([128, NDC, S], BF16, tag="act", name="h2T")
                    for sc in range(NSC):
                        attn_chunk(1, sc, list(range(NTT)), V_sb2, qT2b, kT2b, attnT2)
                    for sc in range(NSC):
                        ln_chunk(h1T, h2T, sc,
                                 lambda dc, g=sc: a_out[1][g // AMERGE][
                                     ts(ag_row(g, dc) // 128, 128), :],
                                 "h2T")

                # ---------- FFN (DFF sharded) + out GEMM + softmax ----------
                # per s-chunk pipeline: FFN(sc) -> yAR -> ln -> out GEMM ->
                # ssAR -> scale+write. outw is loaded up front.
                outT = acts.tile([128, NDC, S], BF16, tag="act", name="outT")
                with tc.tile_pool(name="ffw", bufs=1) as ffp, \
                     tc.tile_pool(name="outwp", bufs=1) as owp, \
                     tc.tile_pool(name="eo", bufs=3) as eop, \
                     tc.tile_pool(name="pp", bufs=3) as ppp:
                    f1w_sb = ffp.tile([128, NDC, FS], BF16, tag="f1w")
                    nc.sync.dma_start(f1w_sb[:, :, :], f1w[:, :, :])
                    f2w_sb = ffp.tile([128, NFC, D], BF16, tag="f2w")
                    nc.sync.dma_start(f2w_sb[:, :, :], f2w[:, :, :])
                    outw_sb = owp.tile([128, NDC, VSP], BF16, tag="outw")
                    for dc in range(NDC):
                        nc.sync.dma_start(outw_sb[:, dc, :], outw[:, dc, :])

                    for sc in range(NSC):
                        uT = ffp.tile([128, NFC, SC], BF16, tag="uT", bufs=2,
                                      name=f"uT_{sc}")
                        for fc in range(NFC):
                            pu = ps.tile([128, SC], F32, tag="ps", name=f"pu_{fc}_{sc}")
                            for dc in range(NDC):
                                nc.tensor.matmul(pu[:, :], f1w_sb[:, dc, ts(fc, 128)],
                                                 h2T[:, dc, ts(sc, SC)],
                                                 start=(dc == 0), stop=(dc == NDC - 1))
                            nc.scalar.activation(uT[:, fc, :], pu[:, :], AF.Relu,
                                                 bias=f1b_sb[:, fc:fc + 1])
                        for dc in range(NDC):
                            py = ps.tile([128, SC], F32, tag="ps", name=f"py_{dc}_{sc}")
                            for fc in range(NFC):
                                nc.tensor.matmul(py[:, :], f2w_sb[:, fc, ts(dc, 128)],
                                                 uT[:, fc, :],
                                                 start=(fc == 0), stop=(fc == NFC - 1))
                            yt = evp.tile([128, SC], BF16, tag="ev")
                            nc.scalar.activation(yt[:, :], py[:, :], AF.Copy)
                            nc.sync.dma_start(y_par[ts(sc * NDC + dc, 128), :], yt[:, :])
                        if (sc + 1) % YMERGE == 0:
                            g0 = sc + 1 - YMERGE
                            nc.gpsimd.collective_compute(
                                "AllReduce", ALU.add, replica_groups=RG,
                                ins=[y_par[g0 * D:(sc + 1) * D, :].opt()],
                                outs=[y_red[sc // YMERGE][:, :].opt()],
                            )

                    for sc in range(NSC):
                        ln_chunk(h2T, outT, sc,
                                 lambda dc, g=sc: y_red[g // YMERGE][
                                     ts((g % YMERGE) * NDC + dc, 128), :],
                                 "outT", extra_bias=f2bT_sb)
                        # ---- out GEMM + exp + col-sums for this chunk ----
                        pss = ps.tile([65, SC], F32, tag="ps", name=f"pss_{sc}")
                        for vc in range(NVC):
                            pl = ps.tile([128, SC], F32, tag="ps", name=f"pl_{sc}_{vc}")
                            for dc in range(NDC):
                                nc.tensor.matmul(pl[:, :], outw_sb[:, dc, ts(vc, 128)],
                                                 outT[:, dc, ts(sc, SC)],
                                                 start=(dc == 0), stop=(dc == NDC - 1))
                            et = eop.tile([128, SC], BF16, tag="eo", name=f"eo_{sc}_{vc}")
                            nc.scalar.activation(et[:, :], pl[:, :], AF.Exp,
                                                 bias=outb_sb[:, vc:vc + 1])
                            nc.tensor.matmul(pss[0:1, :], ones_bf_col[:, :], et[:, :],
                                             start=(vc == 0), stop=(vc == NVC - 1))
                            nc.sync.dma_start(e_dram[ts(vc, 128), ts(sc, SC)], et[:, :])
                        sss = smp.tile([1, SC], F32, tag="nm", name=f"sss_{sc}")
                        nc.scalar.activation(sss[:, :], pss[0:1, :], AF.Copy)
                        nc.sync.dma_start(ss_in[0:1, ts(sc, SC)], sss[:, :])
                        if (sc + 1) % SSMERGE == 0:
                            g0 = sc + 1 - SSMERGE
                            nc.gpsimd.collective_compute(
                                "AllReduce", ALU.add, replica_groups=RG,
                                ins=[ss_in[0:1, g0 * SC:(sc + 1) * SC].opt()],
                                outs=[ss_out[sc // SSMERGE][0:1, :].opt()],
                            )
                            # ---- scale + write probs for the group ----
                            for k in range(g0, sc + 1):
                                rr = smp.tile([1, SC], F32, tag="nm", name=f"rr_{k}")
                                nc.sync.dma_start(
                                    rr[:, :],
                                    ss_out[k // SSMERGE][0:1, ts(k % SSMERGE, SC)])
                                ri = smp.tile([1, SC], F32, tag="inv1", name=f"ri_{k}")
                                nc.vector.reciprocal(ri[:, :], rr[:, :])
                                pr = ps.tile([128, SC], F32, tag="ps", name=f"pr_{k}")
                                nc.tensor.matmul(pr[:, :], ones_row[:, :], ri[:, :],
                                                 start=True, stop=True)
                                recb = bcp.tile([128, SC], BF16, tag="recb",
                                                name=f"recb_{k}")
                                nc.scalar.activation(recb[:, :], pr[:, :], AF.Copy)
                                for vc in range(NVC):
                                    ee = eop.tile([128, SC], BF16, tag="ee",
                                                  name=f"ee_{vc}_{k}")
                                    nc.sync.dma_start(ee[:, :],
                                                      e_dram[ts(vc, 128), ts(k, SC)])
                                    pp = ppp.tile([128, SC], F32, tag="pp",
                                                  name=f"pp_{vc}_{k}")
                                    nc.vector.tensor_mul(pp[:, :], ee[:, :], recb[:, :])
                                    nc.sync.dma_start(probsT[ts(vc, 128), ts(k, SC)],
                                                      pp[:, :])

    nc.compile()
    return nc


def xT_rows(t, kc):
    return t[ts(kc, 128), :]


def _positional_encoding():
    pos = np.arange(S, dtype=np.float32)[:, None]
    i = np.arange(0, D, 2, dtype=np.float32)
    ang = (pos * np.exp((-np.log(10000.0) * i / D).astype(np.float32))).astype(np.float32)
    pe = np.zeros((S, D), np.float32)
    pe[:, 0::2] = np.sin(ang)
    pe[:, 1::2] = np.cos(ang)
    return pe


def _bf(x):
    return np.ascontiguousarray(x).astype(ml_dtypes.bfloat16)


def _f32(x):
    return np.ascontiguousarray(x, dtype=np.float32)


def prepare_inputs(inp):
    """Full fp32 inputs -> per-core input maps (host-side sharding/layout)."""
    li = L - 1
    xT_full = np.ascontiguousarray(inp["x"].T)          # [V, S]
    peb = (inp["emb_b"][None, :] + _positional_encoding()).astype(np.float32)
    pebT = _bf(peb.T)                                    # [D, S] bf16

    # causal mask patterns for the 4 diagonal t-tiles of an s-chunk
    t_loc = np.arange(128)[:, None]
    s_loc = np.arange(SC)[None, :]
    maskT = np.concatenate(
        [((p * 128 + t_loc) <= s_loc).astype(np.float32) for p in range(4)], axis=1
    )
    maskT = _bf(maskT)                                   # [128, 2048]

    in_maps = []
    for c in range(NCORES):
        m = {}
        xs = xT_full[c * VSR:(c + 1) * VSR]              # [4000, S]
        m["xT"] = _bf(np.concatenate([xs, np.zeros((VSP - VSR, S), np.float32)], 0))
        ew = inp["emb_W"][c * VSR:(c + 1) * VSR]
        m["embW"] = _bf(np.concatenate([ew, np.zeros((VSP - VSR, D), np.float32)], 0))
        m["pebT"] = pebT
        m["maskT"] = maskT
        for mi, (Wq, bq, Wk, bk, Wv, bvv) in enumerate([
            (inp["Wq1"][li], inp["bq1"][li], inp["Wk1"][li], inp["bk1"][li],
             inp["Wv1"][li], inp["bv1"][li]),
            (inp["Wq2"][li], inp["bq2"][li], inp["Wk2"][li], inp["bk2"][li],
             inp["Wv2"][li], inp["bv2"][li]),
        ]):
            h0, h1 = 2 * c, 2 * c + 1
            qk = np.concatenate([Wq[h0], Wq[h1], Wk[h0], Wk[h1]], axis=1)  # [D, 256]
            m[f"qkw{mi+1}"] = _bf(qk.reshape(NDC, 128, 256).transpose(1, 0, 2))
            m[f"bqk{mi+1}"] = _f32(np.stack(
                [np.concatenate([bq[h0], bq[h1]]),
                 np.concatenate([bk[h0], bk[h1]])], axis=1))
            vp = np.zeros((D, 130), np.float32)
            vp[:, 0:64] = Wv[h0]
            vp[:, 65:129] = Wv[h1]
            m[f"vw{mi+1}"] = _bf(vp.reshape(NDC, 128, 130).transpose(1, 0, 2))
            m[f"bv{mi+1}"] = _f32(np.concatenate([bvv[h0], bvv[h1]])[:, None])
        w1 = inp["ff_W1"][li][:, c * FS:(c + 1) * FS]    # [D, FS]
        m["f1w"] = _bf(w1.reshape(NDC, 128, FS).transpose(1, 0, 2))
        m["f1b"] = _f32(inp["ff_b1"][li][c * FS:(c + 1) * FS].reshape(NFC, 128).T)
        w2 = inp["ff_W2"][li][c * FS:(c + 1) * FS]       # [FS, D]
        m["f2w"] = _bf(w2.reshape(NFC, 128, D).transpose(1, 0, 2))
        m["f2bT"] = _f32(inp["ff_b2"][li].reshape(NDC, 128).T)
        m["lngT"] = _f32(inp["ln_g"].reshape(NDC, 128).T)
        m["lnbT"] = _f32(inp["ln_b"].reshape(NDC, 128).T)
        ow = inp["out_W"][:, c * VSR:(c + 1) * VSR]      # [D, 4000]
        ow = np.concatenate([ow, np.zeros((D, VSP - VSR), np.float32)], axis=1)
        m["outw"] = _bf(ow.reshape(NDC, 128, VSP).transpose(1, 0, 2))
        ob = np.full(VSP, -30.0, np.float32)
        ob[:VSR] = inp["out_b"][c * VSR:(c + 1) * VSR]
        m["outb"] = _f32(ob.reshape(NVC, 128).T)
        in_maps.append(m)
    return in_maps


_NC_CACHE = {}


def kernel(**inputs):
    inputs = {k: np.asarray(v, dtype=np.float32) for k, v in inputs.items()}
    if "nc" not in _NC_CACHE:
        _NC_CACHE["nc"] = build_bass()
    nc = _NC_CACHE["nc"]
    in_maps = prepare_inputs(inputs)
    res = run_bass_kernel_spmd(nc, in_maps, list(range(NCORES)), trace=False)
    LAST_RESULTS["res"] = res
    shards = [res.results[c]["probsT"][:VSR] for c in range(NCORES)]
    return np.ascontiguousarray(np.concatenate(shards, axis=0).T)


# revision 27
# speedup vs baseline: 1.1727x; 1.1727x over previous
"""Trainium2 Bass kernel for nn_LonelyDecoder (dense transformer, 8-core TP).

Key observations baked in:
 - In the reference, every layer recomputes from the embedding output `h`
   and only the LAST layer's `out` feeds the logits -> layers 0..L-2 are
   dead code. We compute: embedding GEMM, layer L-1, output GEMM+softmax.
 - Scores are tiny ((q.k)/1024, |s| < ~1), so softmax needs no max pass.
 - All activations are kept TRANSPOSED ([feature, seq]) so that:
     * matmul contraction dims land on partitions for both operands,
     * per-feature biases are per-partition (fused into ACT evictions),
     * head-concat == AllGather along the partition axis.

Sharding (8 cores):
 - vocab dim of x/emb_W/out_W (4000/core, padded to 4096)
 - heads of attention (2 heads/core), DFF of the FFN (512/core)

Collective strategy (trn2 ring collectives have a ~10us-per-ring-step
firmware latency floor, so FEWER+LARGER collectives win):
 - exactly one collective per algorithmic step, bf16 payloads;
 - the softmax-denominator reduction uses AllGather (7 ring steps) +
   an on-chip 8-partition PE reduce instead of AllReduce (14 steps);
 - LN is fused per s-chunk (residual+stats+normalize, bf16 vector ops,
   scalar-engine per-partition affine) and the tail (yAR -> LN -> out
   GEMM -> sums -> scale+write) is emitted per chunk so DMA/vector work
   hides under the out GEMM.
"""

import numpy as np
import ml_dtypes

import concourse.bacc as bacc
import concourse.bass as bass
import concourse.mybir as mybir
import concourse.tile as tile
from concourse.bass_utils import run_bass_kernel_spmd

F32 = mybir.dt.float32
BF16 = mybir.dt.bfloat16
AF = mybir.ActivationFunctionType
ALU = mybir.AluOpType

S, V, D, H, DK, DFF, L = 2048, 32000, 1024, 16, 64, 4096, 4
NCORES = 8
VSR = V // NCORES          # 4000 real vocab shard
VSP = 4096                 # padded vocab shard (32 x 128)
NVC = VSP // 128           # 32 v-chunks
NDC = D // 128             # 8 d-chunks
NSC = 4                    # s-chunks of 512
SC = 512
NTT = S // 128             # 16 t-tiles
FS = DFF // NCORES         # 512 ff shard
NFC = FS // 128            # 4 ff chunks
RG = [list(range(NCORES))]

# collective chunking: merge factor in s-chunks (1, 2, or 4); 4 = one
# collective per step (ring-step latency floor dominates, so default 4)
HMERGE = 4                 # embedding-output AllReduce
AMERGE = 4                 # attention-output AllGathers
YMERGE = 4                 # FFN-output AllReduce
SS_USE_AG = True           # softmax sums: AllGather+local reduce vs AllReduce

LAST_RESULTS = {}          # stash for test harness (exec time etc.)


def ts(i, n):
    return slice(i * n, (i + 1) * n)


def build_bass(n_iters=1):
    """n_iters>1 replicates the kernel body inside one NEFF (benchmarking
    only; iterations serialize through reused DRAM scratch + pool slots)."""
    nc = bacc.Bacc(None, target_bir_lowering=False)

    # ---- I/O ----
    xT = nc.dram_tensor("xT", [VSP, S], BF16, kind="ExternalInput")
    embW = nc.dram_tensor("embW", [VSP, D], BF16, kind="ExternalInput")
    pebT = nc.dram_tensor("pebT", [D, S], BF16, kind="ExternalInput")
    qkw = [nc.dram_tensor(f"qkw{m}", [128, NDC, 256], BF16, kind="ExternalInput") for m in (1, 2)]
    bqk = [nc.dram_tensor(f"bqk{m}", [128, 2], F32, kind="ExternalInput") for m in (1, 2)]
    vw = [nc.dram_tensor(f"vw{m}", [128, NDC, 130], BF16, kind="ExternalInput") for m in (1, 2)]
    bv = [nc.dram_tensor(f"bv{m}", [128, 1], F32, kind="ExternalInput") for m in (1, 2)]
    maskT = nc.dram_tensor("maskT", [128, 4 * SC], BF16, kind="ExternalInput")
    f1w = nc.dram_tensor("f1w", [128, NDC, FS], BF16, kind="ExternalInput")
    f1b = nc.dram_tensor("f1b", [128, NFC], F32, kind="ExternalInput")
    f2w = nc.dram_tensor("f2w", [128, NFC, D], BF16, kind="ExternalInput")
    f2bT = nc.dram_tensor("f2bT", [128, NDC], F32, kind="ExternalInput")
    lngT = nc.dram_tensor("lngT", [128, NDC], F32, kind="ExternalInput")
    lnbT = nc.dram_tensor("lnbT", [128, NDC], F32, kind="ExternalInput")
    outw = nc.dram_tensor("outw", [128, NDC, VSP], BF16, kind="ExternalInput")
    outb = nc.dram_tensor("outb", [128, NVC], F32, kind="ExternalInput")
    probsT = nc.dram_tensor("probsT", [VSP, S], F32, kind="ExternalOutput")

    with tile.TileContext(nc) as tc:
        with tc.tile_pool(name="dram", bufs=1, space="DRAM") as dram, \
             tc.tile_pool(name="ps", bufs=8, space="PSUM") as ps, \
             tc.tile_pool(name="const", bufs=1) as const, \
             tc.tile_pool(name="acts", bufs=2) as acts, \
             tc.tile_pool(name="ev", bufs=3) as evp, \
             tc.tile_pool(name="small", bufs=2) as smp, \
             tc.tile_pool(name="addin", bufs=4) as adp, \
             tc.tile_pool(name="x2p", bufs=3) as x2p, \
             tc.tile_pool(name="bcast", bufs=4) as bcp:

            # non-collective DRAM scratch: allocated once, reused by every
            # iteration (the WAR deps serialize bench iterations)
            h_par = dram.tile([NSC * D, SC], BF16, tag="h_par")
            a_in = [dram.tile([NSC * 128, SC], BF16, tag=f"a{m}_in", name=f"a{m}_in") for m in (1, 2)]
            y_par = dram.tile([NSC * D, SC], BF16, tag="y_par")
            e_dram = dram.tile([VSP, S], BF16, tag="e_dram")
            ss_in = dram.tile([1, S], F32, tag="ss_in")

            # constants
            ones_bf_col = const.tile([128, 1], BF16, tag="c1")
            nc.vector.memset(ones_bf_col[:, :], 1.0)
            ones_row = const.tile([1, 128], F32, tag="c3")
            nc.vector.memset(ones_row[:, :], 1.0)
            ones_row64 = const.tile([1, 64], F32, tag="c4")
            nc.vector.memset(ones_row64[:, :], 1.0)
            ones8 = const.tile([8, 1], F32, tag="c8")
            nc.vector.memset(ones8[:, :], 1.0)
            eps_tile = const.tile([1, 1], F32, tag="c5")
            nc.vector.memset(eps_tile[:, :], 1e-5)
            mask_sb = const.tile([128, 4 * SC], BF16, tag="mask")
            nc.sync.dma_start(mask_sb[:, :], maskT[:, :])
            bqk_sb = [const.tile([128, 2], F32, tag=f"bqk{m}", name=f"bqk_sb{m}") for m in range(2)]
            bv_sb = [const.tile([128, 1], F32, tag=f"bv{m}", name=f"bv_sb{m}") for m in range(2)]
            for m in range(2):
                nc.sync.dma_start(bqk_sb[m][:, :], bqk[m][:, :])
                nc.sync.dma_start(bv_sb[m][:, :], bv[m][:, :])
            f1b_sb = const.tile([128, NFC], F32, tag="f1b")
            nc.sync.dma_start(f1b_sb[:, :], f1b[:, :])
            f2bT_sb = const.tile([128, NDC], F32, tag="f2bT")
            nc.sync.dma_start(f2bT_sb[:, :], f2bT[:, :])
            lng_sb = const.tile([128, NDC], F32, tag="lng")
            nc.sync.dma_start(lng_sb[:, :], lngT[:, :])
            lnb_sb = const.tile([128, NDC], F32, tag="lnb")
            nc.sync.dma_start(lnb_sb[:, :], lnbT[:, :])
            outb_sb = const.tile([128, NVC], F32, tag="outb")
            nc.sync.dma_start(outb_sb[:, :], outb[:, :])

            for it in range(n_iters):
                _body(nc, tc, it, dram, ps, const, acts, evp, smp, adp, x2p, bcp,
                      xT, embW, pebT, qkw, vw, maskT, f1w, f2w, outw, probsT,
                      h_par, a_in, y_par, e_dram, ss_in,
                      ones_bf_col, ones_row, ones_row64, ones8, eps_tile,
                      mask_sb, bqk_sb, bv_sb, f1b_sb, f2bT_sb, lng_sb, lnb_sb,
                      outb_sb)

    nc.compile()
    return nc


def _body(nc, tc, it, dram, ps, const, acts, evp, smp, adp, x2p, bcp,
          xT, embW, pebT, qkw, vw, maskT, f1w, f2w, outw, probsT,
          h_par, a_in, y_par, e_dram, ss_in,
          ones_bf_col, ones_row, ones_row64, ones8, eps_tile,
          mask_sb, bqk_sb, bv_sb, f1b_sb, f2bT_sb, lng_sb, lnb_sb, outb_sb):
    X = f"I{it}_"

    # Shared (collective-output) tensors: single writer each -> per iteration
    h_red = [dram.tile([HMERGE * D, SC], BF16, tag=f"{X}h_red{g}",
                       addr_space="Shared", name=f"{X}h_red{g}")
             for g in range(NSC // HMERGE)]
    a_out = [[dram.tile([AMERGE * D, SC], BF16, tag=f"{X}a{m}_out{g}",
                        addr_space="Shared", name=f"{X}a{m}_out{g}")
              for g in range(NSC // AMERGE)] for m in (1, 2)]
    y_red = [dram.tile([YMERGE * D, SC], BF16, tag=f"{X}y_red{g}",
                       addr_space="Shared", name=f"{X}y_red{g}")
             for g in range(NSC // YMERGE)]
    ss_out = dram.tile([8 if SS_USE_AG else 1, S], F32, tag=f"{X}ss_out",
                       addr_space="Shared", name=f"{X}ss_out")

    def ag_row(sc, dc):
        return dc * AMERGE * 128 + (sc % AMERGE) * 128

    # ---------- embedding GEMM:  hT_partial = embW^T @ xT ----------
    with tc.tile_pool(name=f"{X}embw", bufs=1) as embp, \
         tc.tile_pool(name=f"{X}xt", bufs=4) as xtp:
        embw_sb = embp.tile([128, NVC, D], BF16, tag="embw")
        for kc in range(NVC):
            nc.sync.dma_start(embw_sb[:, kc, :], embW[ts(kc, 128), :])
        for sc in range(NSC):
            pes = []
            for dc in range(NDC):
                t = ps.tile([128, SC], F32, tag="ps", name=f"{X}pe_{sc}_{dc}")
                pes.append(t)
            for kc in range(NVC):
                xt = xtp.tile([128, SC], BF16, tag="xt")
                nc.sync.dma_start(xt[:, :], xT[ts(kc, 128), ts(sc, SC)])
                for dc in range(NDC):
                    nc.tensor.matmul(
                        pes[dc][:, :],
                        embw_sb[:, kc, ts(dc, 128)],
                        xt[:, :],
                        start=(kc == 0),
                        stop=(kc == NVC - 1),
                    )
            for dc in range(NDC):
                hv = evp.tile([128, SC], BF16, tag="ev")
                nc.scalar.activation(hv[:, :], pes[dc][:, :], AF.Copy)
                nc.sync.dma_start(h_par[ts(sc * NDC + dc, 128), :], hv[:, :])
            if (sc + 1) % HMERGE == 0:
                g0 = sc + 1 - HMERGE
                nc.gpsimd.collective_compute(
                    "AllReduce", ALU.add, replica_groups=RG,
                    ins=[h_par[g0 * D:(sc + 1) * D, :].opt()],
                    outs=[h_red[sc // HMERGE][:, :].opt()],
                )

    # ======== phase A: attention x2, layernorms ========
    # hT = h_red + (emb_b + PE)^T   -> bf16 resident
    hT = acts.tile([128, NDC, S], BF16, tag="act", name=f"{X}hT")
    for sc in range(NSC):
        for dc in range(NDC):
            hr = adp.tile([128, SC], BF16, tag="addin", name=f"{X}hr_{sc}_{dc}")
            nc.sync.dma_start(
                hr[:, :],
                h_red[sc // HMERGE][ts((sc % HMERGE) * NDC + dc, 128), :])
            pb = adp.tile([128, SC], BF16, tag="addin", name=f"{X}pb_{sc}_{dc}")
            nc.sync.dma_start(pb[:, :], pebT[ts(dc, 128), ts(sc, SC)])
            nc.vector.tensor_add(hT[:, dc, ts(sc, SC)], hr[:, :], pb[:, :])

    # fused residual + layernorm for one s-chunk:
    #   r(sc) = prevT(sc) + addin(sc)   (in place into prevT)
    #   newT(sc) = (r - mu) * inv * g + b
    def ln_chunk(prevT, newT, sc, load_addin, name, extra_bias=None):
        for dc in range(NDC):
            ad = adp.tile([128, SC], BF16, tag="addin",
                          name=f"{X}ad_{name}_{sc}_{dc}")
            nc.sync.dma_start(ad[:, :], load_addin(dc))
            if extra_bias is not None:
                ab = adp.tile([128, SC], BF16, tag="addin",
                              name=f"{X}ab_{name}_{sc}_{dc}")
                nc.scalar.activation(ab[:, :], ad[:, :], AF.Identity,
                                     bias=extra_bias[:, dc:dc + 1])
                ad = ab
            nc.vector.tensor_add(prevT[:, dc, ts(sc, SC)],
                                 prevT[:, dc, ts(sc, SC)], ad[:, :])
        st = ps.tile([65, SC], F32, tag="ps", name=f"{X}st_{name}_{sc}")
        for dc in range(NDC):
            x2 = x2p.tile([128, SC], BF16, tag="x2", name=f"{X}x2_{name}_{sc}_{dc}")
            nc.vector.tensor_mul(x2[:, :], prevT[:, dc, ts(sc, SC)],
                                 prevT[:, dc, ts(sc, SC)])
            nc.tensor.matmul(st[0:1, :], ones_bf_col[:, :],
                             prevT[:, dc, ts(sc, SC)],
                             start=(dc == 0), stop=(dc == NDC - 1))
            nc.tensor.matmul(st[64:65, :], ones_bf_col[:, :], x2[:, :],
                             start=(dc == 0), stop=(dc == NDC - 1))
        nm = smp.tile([1, SC], F32, tag="nm", name=f"{X}nm_{name}_{sc}")
        nc.vector.tensor_scalar_mul(nm[:, :], st[0:1, :], -1.0 / D)
        e2 = smp.tile([1, SC], F32, tag="e2", name=f"{X}e2_{name}_{sc}")
        nc.vector.tensor_scalar_mul(e2[:, :], st[64:65, :], 1.0 / D)
        musq = smp.tile([1, SC], F32, tag="scr2", name=f"{X}mq_{name}_{sc}")
        nc.vector.tensor_mul(musq[:, :], nm[:, :], nm[:, :])
        nc.vector.tensor_sub(e2[:, :], e2[:, :], musq[:, :])
        nc.scalar.activation(e2[:, :], e2[:, :], AF.Sqrt, bias=eps_tile[:, :])
        inv = smp.tile([1, SC], F32, tag="inv1", name=f"{X}inv_{name}_{sc}")
        nc.vector.reciprocal(inv[:, :], e2[:, :])
        nminv = smp.tile([1, SC], F32, tag="scr2", name=f"{X}nmi_{name}_{sc}")
        nc.vector.tensor_mul(nminv[:, :], nm[:, :], inv[:, :])
        # broadcast inv and nm*inv over partitions via PE
        pi = ps.tile([128, SC], F32, tag="ps", name=f"{X}pi_{name}_{sc}")
        nc.tensor.matmul(pi[:, :], ones_row[:, :], inv[:, :], start=True, stop=True)
        inv_b = bcp.tile([128, SC], BF16, tag="bc", name=f"{X}invb_{name}_{sc}")
        nc.scalar.activation(inv_b[:, :], pi[:, :], AF.Copy)
        pm = ps.tile([128, SC], F32, tag="ps", name=f"{X}pm_{name}_{sc}")
        nc.tensor.matmul(pm[:, :], ones_row[:, :], nminv[:, :], start=True, stop=True)
        nmi_b = bcp.tile([128, SC], BF16, tag="bc", name=f"{X}nmib_{name}_{sc}")
        nc.scalar.activation(nmi_b[:, :], pm[:, :], AF.Copy)
        for dc in range(NDC):
            t1 = x2p.tile([128, SC], BF16, tag="x2", name=f"{X}t1_{name}_{sc}_{dc}")
            nc.vector.tensor_mul(t1[:, :], prevT[:, dc, ts(sc, SC)], inv_b[:, :])
            nc.vector.tensor_add(t1[:, :], t1[:, :], nmi_b[:, :])
            nc.scalar.activation(newT[:, dc, ts(sc, SC)], t1[:, :], AF.Identity,
                                 scale=lng_sb[:, dc:dc + 1],
                                 bias=lnb_sb[:, dc:dc + 1])

    with tc.tile_pool(name=f"{X}attn", bufs=1) as attnp:
        qkw_sb = attnp.tile([128, NDC, 256], BF16, tag="qkw", name=f"{X}qkw_sb0")
        nc.sync.dma_start(qkw_sb[:, :, :], qkw[0][:, :, :])
        vw_sb = attnp.tile([128, NDC, 130], BF16, tag="vw", name=f"{X}vw_sb0")
        nc.sync.dma_start(vw_sb[:, :, :], vw[0][:, :, :])
        V_sb = attnp.tile([128, NTT, 130], BF16, tag="V", name=f"{X}V_sb0")
        qT2 = attnp.tile([128, NSC, SC], BF16, tag="qT2", name=f"{X}qT2_0")
        kT2 = attnp.tile([128, NSC, SC], BF16, tag="kT2", name=f"{X}kT2_0")
        attnT = attnp.tile([128, NSC, SC], BF16, tag="attnT", name=f"{X}attnT0")

        def qkv_proj(mi, actT, sc, V_sb, qT2, kT2, qkw_sb, vw_sb):
            for tt in range(4 * sc, 4 * sc + 4):
                pv = ps.tile([128, 130], F32, tag="ps", name=f"{X}pv{mi}_{tt}")
                for dc in range(NDC):
                    nc.tensor.matmul(
                        pv[:, :], actT[:, dc, ts(tt, 128)], vw_sb[:, dc, :],
                        start=(dc == 0), stop=(dc == NDC - 1),
                    )
                nc.scalar.activation(V_sb[:, tt, :], pv[:, :], AF.Copy)
            nc.vector.memset(V_sb[:, ts(sc, 4), 64:65], 1.0)
            nc.vector.memset(V_sb[:, ts(sc, 4), 129:130], 1.0)
            for wi, dst in ((0, qT2), (1, kT2)):
                pq = ps.tile([128, SC], F32, tag="ps", name=f"{X}pq{mi}_{wi}_{sc}")
                for dc in range(NDC):
                    nc.tensor.matmul(
                        pq[:, :],
                        qkw_sb[:, dc, ts(wi, 128)],
                        actT[:, dc, ts(sc, SC)],
                        start=(dc == 0), stop=(dc == NDC - 1),
                    )
                nc.scalar.activation(
                    dst[:, sc, :], pq[:, :], AF.Identity,
                    bias=bqk_sb[mi][:, wi:wi + 1],
                )

        def attn_chunk(mi, sc, tts, V_sb, qT2, kT2, attnT):
            """scores+AV+normalize for query chunk sc over key tiles tts;
            writes attnT[:, sc, :] and a_in chunk, AGs on merge boundary."""
            masked = (mi == 0)
            for h in range(2):
                po = ps.tile([128, SC], F32, tag="ps", name=f"{X}po{mi}_{h}_{sc}")
                for i, tt in enumerate(tts):
                    pscr = ps.tile([128, SC], F32, tag="ps",
                                   name=f"{X}pscr{mi}_{h}_{sc}_{tt}")
                    nc.tensor.matmul(
                        pscr[:, :],
                        kT2[ts(h, 64), tt // 4, ts(tt % 4, 128)],
                        qT2[ts(h, 64), sc, :],
                        start=True, stop=True,
                    )
                    et = evp.tile([128, SC], BF16, tag="exp")
                    nc.scalar.activation(et[:, :], pscr[:, :], AF.Exp, scale=1.0 / D)
                    if masked and tt >= 4 * sc:
                        nc.vector.tensor_mul(
                            et[:, :], et[:, :], mask_sb[:, ts(tt - 4 * sc, SC)],
                        )
                    nc.tensor.matmul(
                        po[0:65, :],
                        V_sb[:, tt, ts(h, 65)],
                        et[:, :],
                        start=(i == 0), stop=(i == len(tts) - 1),
                    )
                rec = attnp.tile([1, SC], F32, tag="rec", bufs=2,
                                 name=f"{X}rec{mi}_{h}_{sc}")
                nc.vector.reciprocal(rec[:, :], po[64:65, :])
                pbv = ps.tile([128, SC], F32, tag="ps", name=f"{X}pb{mi}_{h}_{sc}")
                nc.tensor.matmul(pbv[0:64, :], ones_row64[:, :], rec[:, :],
                                 start=True, stop=True)
                oo = attnp.tile([64, SC], F32, tag="oo", bufs=2,
                                name=f"{X}oo{mi}_{h}_{sc}")
                nc.scalar.activation(oo[:, :], po[0:64, :], AF.Copy)
                tmp = attnp.tile([64, SC], F32, tag="avtmp", bufs=2,
                                 name=f"{X}avtmp{mi}_{h}_{sc}")
                nc.vector.tensor_mul(tmp[:, :], oo[:, :], pbv[0:64, :])
                nc.scalar.activation(
                    attnT[ts(h, 64), sc, :], tmp[:, :], AF.Identity,
                    bias=bv_sb[mi][ts(h, 64), :],
                )
            nc.sync.dma_start(a_in[mi][ts(sc, 128), :], attnT[:, sc, :])
            if (sc + 1) % AMERGE == 0:
                g0 = sc + 1 - AMERGE
                nc.gpsimd.collective_compute(
                    "AllGather", ALU.bypass, replica_groups=RG,
                    ins=[a_in[mi][g0 * 128:(sc + 1) * 128, :].opt()],
                    outs=[a_out[mi][sc // AMERGE][:, :].opt()],
                )

        h1T = acts.tile([128, NDC, S], BF16, tag="act", name=f"{X}h1T")
        for sc in range(NSC):
            qkv_proj(0, hT, sc, V_sb, qT2, kT2, qkw_sb, vw_sb)
            attn_chunk(0, sc, list(range(4 * (sc + 1))), V_sb, qT2, kT2, attnT)
        qkw_sb2 = attnp.tile([128, NDC, 256], BF16, tag="qkw2", name=f"{X}qkw_sb1")
        nc.sync.dma_start(qkw_sb2[:, :, :], qkw[1][:, :, :])
        vw_sb2 = attnp.tile([128, NDC, 130], BF16, tag="vw2", name=f"{X}vw_sb1")
        nc.sync.dma_start(vw_sb2[:, :, :], vw[1][:, :, :])
        V_sb2 = attnp.tile([128, NTT, 130], BF16, tag="V2", name=f"{X}V_sb1")
        qT2b = attnp.tile([128, NSC, SC], BF16, tag="qT2b", name=f"{X}qT2_1")
        kT2b = attnp.tile([128, NSC, SC], BF16, tag="kT2b", name=f"{X}kT2_1")
        attnT2 = attnp.tile([128, NSC, SC], BF16, tag="attnT2", name=f"{X}attnT1")
        for sc in range(NSC):
            ln_chunk(hT, h1T, sc,
                     lambda dc, g=sc: a_out[0][g // AMERGE][
                         ts(ag_row(g, dc) // 128, 128), :],
                     "h1T")
            qkv_proj(1, h1T, sc, V_sb2, qT2b, kT2b, qkw_sb2, vw_sb2)

        h2T = acts.tile([128, NDC, S], BF16, tag="act", name=f"{X}h2T")
        for sc in range(NSC):
            attn_chunk(1, sc, list(range(NTT)), V_sb2, qT2b, kT2b, attnT2)
        for sc in range(NSC):
            ln_chunk(h1T, h2T, sc,
                     lambda dc, g=sc: a_out[1][g // AMERGE][
                         ts(ag_row(g, dc) // 128, 128), :],
                     "h2T")

    # ---------- FFN (DFF sharded) + out GEMM + softmax ----------
    outT = acts.tile([128, NDC, S], BF16, tag="act", name=f"{X}outT")
    with tc.tile_pool(name=f"{X}ffw", bufs=1) as ffp, \
         tc.tile_pool(name=f"{X}outwp", bufs=1) as owp, \
         tc.tile_pool(name=f"{X}eo", bufs=3) as eop, \
         tc.tile_pool(name=f"{X}pp", bufs=2) as ppp:
        f1w_sb = ffp.tile([128, NDC, FS], BF16, tag="f1w")
        nc.sync.dma_start(f1w_sb[:, :, :], f1w[:, :, :])
        f2w_sb = ffp.tile([128, NFC, D], BF16, tag="f2w")
        nc.sync.dma_start(f2w_sb[:, :, :], f2w[:, :, :])
        outw_sb = owp.tile([128, NDC, VSP], BF16, tag="outw")
        for dc in range(NDC):
            nc.sync.dma_start(outw_sb[:, dc, :], outw[:, dc, :])

        for sc in range(NSC):
            uT = ffp.tile([128, NFC, SC], BF16, tag="uT", bufs=2, name=f"{X}uT_{sc}")
            for fc in range(NFC):
                pu = ps.tile([128, SC], F32, tag="ps", name=f"{X}pu_{fc}_{sc}")
                for dc in range(NDC):
                    nc.tensor.matmul(pu[:, :], f1w_sb[:, dc, ts(fc, 128)],
                                     h2T[:, dc, ts(sc, SC)],
                                     start=(dc == 0), stop=(dc == NDC - 1))
                nc.scalar.activation(uT[:, fc, :], pu[:, :], AF.Relu,
                                     bias=f1b_sb[:, fc:fc + 1])
            for dc in range(NDC):
                py = ps.tile([128, SC], F32, tag="ps", name=f"{X}py_{dc}_{sc}")
                for fc in range(NFC):
                    nc.tensor.matmul(py[:, :], f2w_sb[:, fc, ts(dc, 128)],
                                     uT[:, fc, :],
                                     start=(fc == 0), stop=(fc == NFC - 1))
                yt = evp.tile([128, SC], BF16, tag="ev")
                nc.scalar.activation(yt[:, :], py[:, :], AF.Copy)
                nc.sync.dma_start(y_par[ts(sc * NDC + dc, 128), :], yt[:, :])
            if (sc + 1) % YMERGE == 0:
                g0 = sc + 1 - YMERGE
                nc.gpsimd.collective_compute(
                    "AllReduce", ALU.add, replica_groups=RG,
                    ins=[y_par[g0 * D:(sc + 1) * D, :].opt()],
                    outs=[y_red[sc // YMERGE][:, :].opt()],
                )

        for sc in range(NSC):
            ln_chunk(h2T, outT, sc,
                     lambda dc, g=sc: y_red[g // YMERGE][
                         ts((g % YMERGE) * NDC + dc, 128), :],
                     "outT", extra_bias=f2bT_sb)
            # ---- out GEMM + exp + col-sums for this chunk ----
            pss = ps.tile([65, SC], F32, tag="ps", name=f"{X}pss_{sc}")
            for vc in range(NVC):
                pl = ps.tile([128, SC], F32, tag="ps", name=f"{X}pl_{sc}_{vc}")
                for dc in range(NDC):
                    nc.tensor.matmul(pl[:, :], outw_sb[:, dc, ts(vc, 128)],
                                     outT[:, dc, ts(sc, SC)],
                                     start=(dc == 0), stop=(dc == NDC - 1))
                et = eop.tile([128, SC], BF16, tag="eo", name=f"{X}eo_{sc}_{vc}")
                nc.scalar.activation(et[:, :], pl[:, :], AF.Exp,
                                     bias=outb_sb[:, vc:vc + 1])
                nc.tensor.matmul(pss[0:1, :], ones_bf_col[:, :], et[:, :],
                                 start=(vc == 0), stop=(vc == NVC - 1))
                nc.sync.dma_start(e_dram[ts(vc, 128), ts(sc, SC)], et[:, :])
            sss = smp.tile([1, SC], F32, tag="nm", name=f"{X}sss_{sc}")
            nc.scalar.activation(sss[:, :], pss[0:1, :], AF.Copy)
            nc.sync.dma_start(ss_in[0:1, ts(sc, SC)], sss[:, :])

        # ---- global softmax sums:  AllGather + on-chip reduce ----
        if SS_USE_AG:
            nc.gpsimd.collective_compute(
                "AllGather", ALU.bypass, replica_groups=RG,
                ins=[ss_in[0:1, :].opt()], outs=[ss_out[:, :].opt()],
            )
        else:
            nc.gpsimd.collective_compute(
                "AllReduce", ALU.add, replica_groups=RG,
                ins=[ss_in[0:1, :].opt()], outs=[ss_out[:, :].opt()],
            )
        for sc in range(NSC):
            ri = smp.tile([1, SC], F32, tag="inv1", name=f"{X}ri_{sc}")
            if SS_USE_AG:
                ssg = smp.tile([8, SC], F32, tag="ssg", bufs=1, name=f"{X}ssg_{sc}")
                nc.sync.dma_start(ssg[:, :], ss_out[:, ts(sc, SC)])
                psr = ps.tile([1, SC], F32, tag="ps", name=f"{X}psr_{sc}")
                nc.tensor.matmul(psr[:, :], ones8[:, :], ssg[:, :],
                                 start=True, stop=True)
                nc.vector.reciprocal(ri[:, :], psr[0:1, :])
            else:
                rr = smp.tile([1, SC], F32, tag="nm", name=f"{X}rr_{sc}")
                nc.sync.dma_start(rr[:, :], ss_out[0:1, ts(sc, SC)])
                nc.vector.reciprocal(ri[:, :], rr[:, :])
            pr = ps.tile([128, SC], F32, tag="ps", name=f"{X}pr_{sc}")
            nc.tensor.matmul(pr[:, :], ones_row[:, :], ri[:, :],
                             start=True, stop=True)
            recb = bcp.tile([128, SC], BF16, tag="recb", name=f"{X}recb_{sc}")
            nc.scalar.activation(recb[:, :], pr[:, :], AF.Copy)
            for vc in range(NVC):
                ee = eop.tile([128, SC], BF16, tag="ee", name=f"{X}ee_{vc}_{sc}")
                nc.sync.dma_start(ee[:, :], e_dram[ts(vc, 128), ts(sc, SC)])
                pp = ppp.tile([128, SC], F32, tag="pp", name=f"{X}pp_{vc}_{sc}")
                nc.vector.tensor_mul(pp[:, :], ee[:, :], recb[:, :])
                nc.sync.dma_start(probsT[ts(vc, 128), ts(sc, SC)], pp[:, :])


def _positional_encoding():
    pos = np.arange(S, dtype=np.float32)[:, None]
    i = np.arange(0, D, 2, dtype=np.float32)
    ang = (pos * np.exp((-np.log(10000.0) * i / D).astype(np.float32))).astype(np.float32)
    pe = np.zeros((S, D), np.float32)
    pe[:, 0::2] = np.sin(ang)
    pe[:, 1::2] = np.cos(ang)
    return pe


def _bf(x):
    return np.ascontiguousarray(x).astype(ml_dtypes.bfloat16)


def _f32(x):
    return np.ascontiguousarray(x, dtype=np.float32)


def prepare_inputs(inp):
    """Full fp32 inputs -> per-core input maps (host-side sharding/layout)."""
    li = L - 1
    xT_full = np.ascontiguousarray(inp["x"].T)          # [V, S]
    peb = (inp["emb_b"][None, :] + _positional_encoding()).astype(np.float32)
    pebT = _bf(peb.T)                                    # [D, S] bf16

    # causal mask patterns for the 4 diagonal t-tiles of an s-chunk
    t_loc = np.arange(128)[:, None]
    s_loc = np.arange(SC)[None, :]
    maskT = np.concatenate(
        [((p * 128 + t_loc) <= s_loc).astype(np.float32) for p in range(4)], axis=1
    )
    maskT = _bf(maskT)                                   # [128, 2048]

    in_maps = []
    for c in range(NCORES):
        m = {}
        xs = xT_full[c * VSR:(c + 1) * VSR]              # [4000, S]
        m["xT"] = _bf(np.concatenate([xs, np.zeros((VSP - VSR, S), np.float32)], 0))
        ew = inp["emb_W"][c * VSR:(c + 1) * VSR]
        m["embW"] = _bf(np.concatenate([ew, np.zeros((VSP - VSR, D), np.float32)], 0))
        m["pebT"] = pebT
        m["maskT"] = maskT
        for mi, (Wq, bq, Wk, bk, Wv, bvv) in enumerate([
            (inp["Wq1"][li], inp["bq1"][li], inp["Wk1"][li], inp["bk1"][li],
             inp["Wv1"][li], inp["bv1"][li]),
            (inp["Wq2"][li], inp["bq2"][li], inp["Wk2"][li], inp["bk2"][li],
             inp["Wv2"][li], inp["bv2"][li]),
        ]):
            h0, h1 = 2 * c, 2 * c + 1
            qk = np.concatenate([Wq[h0], Wq[h1], Wk[h0], Wk[h1]], axis=1)  # [D, 256]
            m[f"qkw{mi+1}"] = _bf(qk.reshape(NDC, 128, 256).transpose(1, 0, 2))
            m[f"bqk{mi+1}"] = _f32(np.stack(
                [np.concatenate([bq[h0], bq[h1]]),
                 np.concatenate([bk[h0], bk[h1]])], axis=1))
            vp = np.zeros((D, 130), np.float32)
            vp[:, 0:64] = Wv[h0]
            vp[:, 65:129] = Wv[h1]
            m[f"vw{mi+1}"] = _bf(vp.reshape(NDC, 128, 130).transpose(1, 0, 2))
            m[f"bv{mi+1}"] = _f32(np.concatenate([bvv[h0], bvv[h1]])[:, None])
        w1 = inp["ff_W1"][li][:, c * FS:(c + 1) * FS]    # [D, FS]
        m["f1w"] = _bf(w1.reshape(NDC, 128, FS).transpose(1, 0, 2))
        m["f1b"] = _f32(inp["ff_b1"][li][c * FS:(c + 1) * FS].reshape(NFC, 128).T)
        w2 = inp["ff_W2"][li][c * FS:(c + 1) * FS]       # [FS, D]
        m["f2w"] = _bf(w2.reshape(NFC, 128, D).transpose(1, 0, 2))
        m["f2bT"] = _f32(inp["ff_b2"][li].reshape(NDC, 128).T)
        m["lngT"] = _f32(inp["ln_g"].reshape(NDC, 128).T)
        m["lnbT"] = _f32(inp["ln_b"].reshape(NDC, 128).T)
        ow = inp["out_W"][:, c * VSR:(c + 1) * VSR]      # [D, 4000]
        ow = np.concatenate([ow, np.zeros((D, VSP - VSR), np.float32)], axis=1)
        m["outw"] = _bf(ow.reshape(NDC, 128, VSP).transpose(1, 0, 2))
        ob = np.full(VSP, -30.0, np.float32)
        ob[:VSR] = inp["out_b"][c * VSR:(c + 1) * VSR]
        m["outb"] = _f32(ob.reshape(NVC, 128).T)
        in_maps.append(m)
    return in_maps


_NC_CACHE = {}


def kernel(**inputs):
    inputs = {k: np.asarray(v, dtype=np.float32) for k, v in inputs.items()}
    if "nc" not in _NC_CACHE:
        _NC_CACHE["nc"] = build_bass()
    nc = _NC_CACHE["nc"]
    in_maps = prepare_inputs(inputs)
    res = run_bass_kernel_spmd(nc, in_maps, list(range(NCORES)), trace=False)
    LAST_RESULTS["res"] = res
    shards = [res.results[c]["probsT"][:VSR] for c in range(NCORES)]
    return np.ascontiguousarray(np.concatenate(shards, axis=0).T)
